# revision 1
# baseline (speedup 1.0000x reference)
"""DeepSeek-V3-style MoE layer on 8 Trainium2 NeuronCores.

Strategy (expert-parallel + shared-expert tensor-parallel):
  - Router (sigmoid over rand_logits, top-4, capacity drop) runs on host:
    it is O(T*E) index math that determines the dispatch, i.e. the sharding.
  - The 32 experts are placed 4-per-core, load-balanced so that every core
    runs an identical (SPMD) instruction stream with static per-slot token
    capacities derived from the actual routing counts.
  - Each core computes its experts' SwiGLU FFN over the tokens routed to
    them, plus a 1/8 slice (intermediate dim) of the shared expert.
  - Host gathers per-assignment rows, applies routing weights, and reduces
    the 8 shared-expert partials: out = scatter(top * y) + sum_c ysh_c.

All matmuls run on the tensor engine with fp16 operands (fp32 PSUM
accumulation) by default; set BASSMOE_DT=f32r for float32r operands.
"""

import functools
import os
import sys
import time

import numpy as np

for _p in ('/opt/trn_rl_repo', '/root/.axon_site/_ro/trn_rl_repo'):
    if os.path.isdir(_p) and _p not in sys.path:
        sys.path.insert(0, _p)

import concourse.bass as bass  # noqa: F401  (AP helpers)
import concourse.tile as tile
from concourse import bacc, mybir
from concourse.bass_utils import run_bass_kernel_spmd

# ---- problem config (hardcoded from spec) ----
T = 2048
D = 2048          # hidden
M = 1408          # expert intermediate
E = 32            # experts
K = 4             # top_k
CAP = 512         # per-expert capacity
ROUTE_SCALE = 2.5
MS = 2816         # shared intermediate (M * 2)
N_CORES = 8
NSLOT = E // N_CORES          # 4 experts per core
MS_LOC = MS // N_CORES        # 352
MS_PAD = 384                  # padded to 3 x 128
KT = D // 128                 # 16 contraction tiles over hidden
MT = M // 128                 # 11 intermediate tiles
DC = D // 512                 # 4 output column chunks of 512

_DT_NAME = os.environ.get("BASSMOE_DT", "f16")
if _DT_NAME == "f16":
    DT, NP_DT, MIN_CAP = mybir.dt.float16, np.float16, 32
elif _DT_NAME == "bf16":
    DT, NP_DT, MIN_CAP = mybir.dt.bfloat16, None, 32
else:  # f32r
    DT, NP_DT, MIN_CAP = mybir.dt.float32, np.float32, 256

if _DT_NAME == "bf16":
    import ml_dtypes
    NP_DT = np.dtype(ml_dtypes.bfloat16)

F32 = mybir.dt.float32
SILU = mybir.ActivationFunctionType.Silu


def _mm_ops(lhsT, rhs):
    if _DT_NAME == "f32r":
        return lhsT.bitcast(mybir.dt.float32r), rhs.bitcast(mybir.dt.float32r)
    return lhsT, rhs


# --------------------------------------------------------------------------
# host-side routing
# --------------------------------------------------------------------------

def _route(rand_logits, expert_bias):
    scores = (1.0 / (1.0 + np.exp(-rand_logits.astype(np.float32)))).astype(np.float32)
    biased = scores + expert_bias[None, :]
    idx = np.argsort(-biased, axis=1, kind="stable")[:, :K]          # [T, K]
    top = np.take_along_axis(scores, idx, axis=1)
    top = top / (top.sum(-1, keepdims=True) + 1e-20) * ROUTE_SCALE   # [T, K]

    flat_e = idx.reshape(-1)
    order = np.argsort(flat_e, kind="stable")                        # assignment ids by expert
    counts = np.bincount(flat_e, minlength=E)
    kept = np.minimum(counts, CAP)
    starts = np.concatenate([[0], np.cumsum(counts)])[:E]
    assigns = [order[starts[e]: starts[e] + kept[e]] for e in range(E)]
    return top, assigns, kept


def _placement(kept):
    """Experts -> (slot, core) grid with uniform per-slot capacities."""
    rank = np.argsort(-kept, kind="stable")
    slots = np.empty((NSLOT, N_CORES), dtype=int)
    caps = []
    for j in range(NSLOT):
        octile = rank[j * N_CORES: (j + 1) * N_CORES]
        if j % 2 == 1:
            octile = octile[::-1]
        slots[j] = octile
        cap = int(((int(kept[octile].max()) + 15) // 16) * 16)
        caps.append(min(max(cap, MIN_CAP), CAP))
    return slots, tuple(caps)


# --------------------------------------------------------------------------
# device program
# --------------------------------------------------------------------------

@functools.lru_cache(maxsize=4)
def _program(caps):
    capsum = sum(caps)
    offs = [0]
    for c in caps:
        offs.append(offs[-1] + c)

    nc = bacc.Bacc("TRN2", target_bir_lowering=False, debug=False,
                   num_devices=N_CORES)
    ap = {}
    ap["xt"] = nc.dram_tensor("xt", [KT, 128, capsum], DT, kind="ExternalInput").ap()
    ap["xts"] = nc.dram_tensor("xts", [KT, 128, T], DT, kind="ExternalInput").ap()
    ap["wg"] = nc.dram_tensor("wg", [NSLOT, MT, 128, KT * 128], DT, kind="ExternalInput").ap()
    ap["wu"] = nc.dram_tensor("wu", [NSLOT, MT, 128, KT * 128], DT, kind="ExternalInput").ap()
    ap["wd"] = nc.dram_tensor("wd", [NSLOT, MT, 128, D], DT, kind="ExternalInput").ap()
    ap["swg"] = nc.dram_tensor("swg", [3, 128, KT * 128], DT, kind="ExternalInput").ap()
    ap["swu"] = nc.dram_tensor("swu", [3, 128, KT * 128], DT, kind="ExternalInput").ap()
    ap["swd"] = nc.dram_tensor("swd", [3, 128, D], DT, kind="ExternalInput").ap()
    ap["ident"] = nc.dram_tensor("ident", [128, 128], DT, kind="ExternalInput").ap()
    ap["yr"] = nc.dram_tensor("yr", [capsum, D], F32, kind="ExternalOutput").ap()
    ap["ysh"] = nc.dram_tensor("ysh", [T, D], F32, kind="ExternalOutput").ap()

    with tile.TileContext(nc) as tc:
        with tc.tile_pool(name="xtp", bufs=2) as xtp, \
             tc.tile_pool(name="wp", bufs=6) as wp, \
             tc.tile_pool(name="hp", bufs=2) as hp, \
             tc.tile_pool(name="wdp", bufs=4) as wdp, \
             tc.tile_pool(name="ytp", bufs=3) as ytp, \
             tc.tile_pool(name="actp", bufs=3) as actp, \
             tc.tile_pool(name="obp", bufs=8) as obp, \
             tc.tile_pool(name="swp", bufs=1) as swp, \
             tc.tile_pool(name="xsp", bufs=2) as xsp, \
             tc.tile_pool(name="hsp", bufs=2) as hsp, \
             tc.tile_pool(name="psgu", bufs=3, space="PSUM") as psgu, \
             tc.tile_pool(name="psy", bufs=2, space="PSUM") as psy:

            def psum_to_sbuf_to_dram(ps_ap, dram_ap, rows):
                ob = obp.tile([128, 512], F32, name="ob", tag="ob")
                nc.vector.tensor_copy(ob[:rows, :], ps_ap)
                nc.sync.dma_start(dram_ap, ob[:rows, :])

            # Shared-expert weights + first token chunk are emitted at slot
            # boundaries (see loop tail) so their DMAs issue well before the
            # shared phase without delaying slot 0's critical-path loads.
            swg_sb = swp.tile([128, 3, KT * 128], DT, name="swg_sb")
            swu_sb = swp.tile([128, 3, KT * 128], DT, name="swu_sb")
            swd_sb = swp.tile([128, 3, D], DT, name="swd_sb")
            xts0_sb = xsp.tile([128, KT, 512], DT, name="xts_sb", tag="xts")
            ident_sb = swp.tile([128, 128], DT, name="ident_sb")

            # ---------------- routed experts ----------------
            prefetched = {}   # j -> (xt_sb, wg0_sb, wu0_sb), loaded mid-slot j-1
            for j, cap in enumerate(caps):
                xt_src = ap["xt"].transpose([1, 0, 2])[:, :, offs[j]: offs[j] + cap]
                if j in prefetched:
                    xt_sb, pre_wg0, pre_wu0 = prefetched.pop(j)
                else:
                    pre_wg0 = pre_wu0 = None
                    xt_sb = xtp.tile([128, KT, cap], DT, name="xt_sb", tag="xt")
                    # first-needed-first: k-tiles 0-3 of tokens + the first
                    # half of gate/up weights land before the bulk remainder
                    nc.sync.dma_start(xt_sb[:, :4, :], xt_src[:, :4, :])

                ht = hp.tile([128, MT, cap], DT, name="ht", tag="ht")
                for m in range(MT):
                    if m == 0 and pre_wg0 is not None:
                        wg_sb, wu_sb = pre_wg0, pre_wu0
                    else:
                        wg_sb = wp.tile([128, KT * 128], DT, name="wg_sb", tag="w")
                        wu_sb = wp.tile([128, KT * 128], DT, name="wu_sb", tag="w")
                        if j == 0 and m == 0:
                            nc.sync.dma_start(wg_sb[:, :512], ap["wg"][j, m, :, :512])
                            nc.sync.dma_start(wu_sb[:, :512], ap["wu"][j, m, :, :512])
                            nc.sync.dma_start(xt_sb[:, 4:, :], xt_src[:, 4:, :])
                            nc.sync.dma_start(wg_sb[:, 512:], ap["wg"][j, m, :, 512:])
                            nc.sync.dma_start(wu_sb[:, 512:], ap["wu"][j, m, :, 512:])
                        else:
                            nc.sync.dma_start(wg_sb[:], ap["wg"][j, m])
                            nc.sync.dma_start(wu_sb[:], ap["wu"][j, m])
                    if m == 5:
                        if j == 0:
                            nc.sync.dma_start(ident_sb[:], ap["ident"])
                        if j + 1 < NSLOT:
                            ncap = caps[j + 1]
                            nxt = xtp.tile([128, KT, ncap], DT, name="xt_sb", tag="xt")
                            nc.sync.dma_start(
                                nxt[:], ap["xt"].transpose([1, 0, 2])
                                [:, :, offs[j + 1]: offs[j + 1] + ncap])
                            nwg = wp.tile([128, KT * 128], DT, name="wg_sb", tag="w")
                            nc.sync.dma_start(nwg[:], ap["wg"][j + 1, 0])
                            nwu = wp.tile([128, KT * 128], DT, name="wu_sb", tag="w")
                            nc.sync.dma_start(nwu[:], ap["wu"][j + 1, 0])
                            prefetched[j + 1] = (nxt, nwg, nwu)
                        else:
                            nc.sync.dma_start(
                                xts0_sb[:],
                                ap["xts"].transpose([1, 0, 2])[:, :, 0:512])

                    psg = psgu.tile([128, cap], F32, name="psg", tag="psgu")
                    for t in range(KT):
                        l, r = _mm_ops(wg_sb[:, t * 128:(t + 1) * 128], xt_sb[:, t, :])
                        nc.tensor.matmul(psg[:], l, r, start=(t == 0), stop=(t == KT - 1))
                    psu = psgu.tile([128, cap], F32, name="psu", tag="psgu")
                    for t in range(KT):
                        l, r = _mm_ops(wu_sb[:, t * 128:(t + 1) * 128], xt_sb[:, t, :])
                        nc.tensor.matmul(psu[:], l, r, start=(t == 0), stop=(t == KT - 1))

                    sact = actp.tile([128, cap], F32, name="sact", tag="act")
                    nc.scalar.activation(sact[:], psg[:], SILU)
                    nc.vector.tensor_mul(ht[:, m, :], sact[:], psu[:])

                # Down-projection, transposed: tokens ride the matmul free dim
                # (cost ∝ cap, not ceil(cap/128)*128), then cheap fp16 PE
                # transposes restore token-major layout for the output.
                nchunk = (cap + 127) // 128
                for g in range(DC):
                    wd_g = wdp.tile([128, MT, 512], DT, name="wd_g", tag="wd")
                    nc.sync.dma_start(
                        wd_g[:],
                        ap["wd"][j].transpose([1, 0, 2])[:, :, g * 512:(g + 1) * 512])
                    if j == NSLOT - 1:
                        # slot 3's down phase is the only stretch with DMA
                        # slack before the shared phase: stage its loads here
                        if g == 0:
                            nc.sync.dma_start(
                                swg_sb[:], ap["swg"].transpose([1, 0, 2]))
                        elif g == 1:
                            nc.sync.dma_start(
                                swu_sb[:], ap["swu"].transpose([1, 0, 2]))
                        elif g == 2:
                            nc.sync.dma_start(
                                swd_sb[:], ap["swd"].transpose([1, 0, 2]))
                    obs = [obp.tile([128, 512], F32, name="ob_td", tag="ob")
                           for _ in range(nchunk)]
                    for k in range(4):
                        ps_yt = psy.tile([128, cap], F32, name="ps_yt", tag="psy")
                        for m in range(MT):
                            l, r = _mm_ops(
                                wd_g[:, m, k * 128:(k + 1) * 128],
                                ht[:, m, :])
                            nc.tensor.matmul(ps_yt[:], l, r,
                                             start=(m == 0), stop=(m == MT - 1))
                        yt_sb = ytp.tile([128, cap], DT, name="yt_sb", tag="yt")
                        nc.vector.tensor_copy(yt_sb[:], ps_yt[:])
                        for cchunk in range(nchunk):
                            rows = min(128, cap - cchunk * 128)
                            ps_t = psy.tile([128, 128], DT, name="ps_t",
                                            tag="pst", bufs=3)
                            nc.tensor.transpose(
                                ps_t[:rows, :],
                                yt_sb[:, cchunk * 128: cchunk * 128 + rows],
                                ident_sb[:])
                            nc.scalar.copy(
                                obs[cchunk][:rows, k * 128:(k + 1) * 128],
                                ps_t[:rows, :])
                    for cchunk in range(nchunk):
                        rows = min(128, cap - cchunk * 128)
                        nc.sync.dma_start(
                            ap["yr"][offs[j] + cchunk * 128: offs[j] + cchunk * 128 + rows,
                                     g * 512:(g + 1) * 512],
                            obs[cchunk][:rows, :])


            # ---------------- shared expert (this core's MS slice) ----------
            for tci in range(T // 512):
                if tci == 0:
                    xts_sb = xts0_sb
                else:
                    xts_sb = xsp.tile([128, KT, 512], DT, name="xts_sb", tag="xts")
                    nc.sync.dma_start(
                        xts_sb[:],
                        ap["xts"].transpose([1, 0, 2])[:, :, tci * 512:(tci + 1) * 512])

                hs = hsp.tile([128, 3, 512], DT, name="hs", tag="hs")
                for m in range(3):
                    psg = psgu.tile([128, 512], F32, name="psg_s", tag="psgu")
                    for t in range(KT):
                        l, r = _mm_ops(swg_sb[:, m, t * 128:(t + 1) * 128], xts_sb[:, t, :])
                        nc.tensor.matmul(psg[:], l, r, start=(t == 0), stop=(t == KT - 1))
                    psu = psgu.tile([128, 512], F32, name="psu_s", tag="psgu")
                    for t in range(KT):
                        l, r = _mm_ops(swu_sb[:, m, t * 128:(t + 1) * 128], xts_sb[:, t, :])
                        nc.tensor.matmul(psu[:], l, r, start=(t == 0), stop=(t == KT - 1))
                    sact = actp.tile([128, 512], F32, name="sact_s", tag="act")
                    nc.scalar.activation(sact[:], psg[:], SILU)
                    nc.vector.tensor_mul(hs[:, m, :], sact[:], psu[:])

                for d in range(DC):
                    for cchunk in range(4):
                        ps = psy.tile([128, 512], F32, name="ps_s", tag="pst",
                                      bufs=3)
                        for m in range(3):
                            l, r = _mm_ops(hs[:, m, cchunk * 128:(cchunk + 1) * 128],
                                           swd_sb[:, m, d * 512:(d + 1) * 512])
                            nc.tensor.matmul(ps[:], l, r, start=(m == 0), stop=(m == 2))
                        psum_to_sbuf_to_dram(
                            ps[:],
                            ap["ysh"][tci * 512 + cchunk * 128: tci * 512 + (cchunk + 1) * 128,
                                      d * 512:(d + 1) * 512],
                            128)
    nc.compile()
    return nc


# --------------------------------------------------------------------------
# host-side packing + combine
# --------------------------------------------------------------------------

def _pack_gu(w):
    # [D, M] -> [MT, 128(k-part), KT*128] stationary-ready layout
    return np.ascontiguousarray(
        w.reshape(KT, 128, MT, 128).transpose(2, 1, 0, 3).reshape(MT, 128, KT * 128))


def kernel(**inputs):
    x = np.asarray(inputs["x"], np.float32)
    rand_logits = np.asarray(inputs["rand_logits"], np.float32)
    expert_bias = np.asarray(inputs["expert_bias"], np.float32)
    wg = np.asarray(inputs["w_gate"], np.float32)
    wu = np.asarray(inputs["w_up"], np.float32)
    wd = np.asarray(inputs["w_down"], np.float32)
    swg = np.asarray(inputs["sw_gate"], np.float32)
    swu = np.asarray(inputs["sw_up"], np.float32)
    swd = np.asarray(inputs["sw_down"], np.float32)

    top, assigns, kept = _route(rand_logits, expert_bias)
    slots, caps = _placement(kept)
    capsum = sum(caps)
    offs = np.concatenate([[0], np.cumsum(caps)]).astype(int)

    global _last_caps
    _last_caps = caps
    t0 = time.time()
    nc = _program(caps)
    t1 = time.time()

    # pack per-core inputs
    xT = np.ascontiguousarray(x.T.astype(NP_DT))                    # [D, T]
    xts3 = xT.reshape(KT, 128, T)
    swg_pad = np.zeros((D, MS_PAD), np.float32)
    swu_pad = np.zeros((D, MS_PAD), np.float32)
    swd_pad = np.zeros((MS_PAD, D), np.float32)

    in_maps = []
    for c in range(N_CORES):
        xt = np.zeros((D, capsum), NP_DT)
        for j in range(NSLOT):
            e = slots[j][c]
            tok = assigns[e] // K
            if len(tok):
                xt[:, offs[j]: offs[j] + len(tok)] = x[tok].astype(NP_DT).T
        wgx = np.stack([_pack_gu(wg[slots[j][c]]) for j in range(NSLOT)])
        wux = np.stack([_pack_gu(wu[slots[j][c]]) for j in range(NSLOT)])
        wdx = np.stack([wd[slots[j][c]].reshape(MT, 128, D) for j in range(NSLOT)])

        swg_pad[:, :MS_LOC] = swg[:, c * MS_LOC:(c + 1) * MS_LOC]
        swu_pad[:, :MS_LOC] = swu[:, c * MS_LOC:(c + 1) * MS_LOC]
        swd_pad[:MS_LOC, :] = swd[c * MS_LOC:(c + 1) * MS_LOC, :]
        swgx = np.ascontiguousarray(
            swg_pad.reshape(KT, 128, 3, 128).transpose(2, 1, 0, 3).reshape(3, 128, KT * 128))
        swux = np.ascontiguousarray(
            swu_pad.reshape(KT, 128, 3, 128).transpose(2, 1, 0, 3).reshape(3, 128, KT * 128))
        swdx = swd_pad.reshape(3, 128, D)

        in_maps.append({
            "xt": xt.reshape(KT, 128, capsum),
            "xts": xts3,
            "ident": np.eye(128, dtype=np.float16) if NP_DT == np.float16
                     else np.eye(128, dtype=NP_DT),
            "wg": wgx.astype(NP_DT),
            "wu": wux.astype(NP_DT),
            "wd": wdx.astype(NP_DT),
            "swg": swgx.astype(NP_DT),
            "swu": swux.astype(NP_DT),
            "swd": swdx.astype(NP_DT),
        })

    t2 = time.time()
    res = run_bass_kernel_spmd(nc, in_maps, core_ids=list(range(N_CORES)))
    t3 = time.time()
    if os.environ.get("BASSMOE_VERBOSE"):
        print(f"[kernel] program build {t1 - t0:.2f}s  pack {t2 - t1:.2f}s  "
              f"device run {t3 - t2:.2f}s", file=sys.stderr)
    outs = res.results

    out = np.zeros((T, D), np.float32)
    for c in range(N_CORES):
        out += outs[c]["ysh"]

    ytk = np.zeros((T, K, D), np.float32)
    for c in range(N_CORES):
        yr = outs[c]["yr"]
        for j in range(NSLOT):
            e = slots[j][c]
            a = assigns[e]
            if len(a):
                ytk[a // K, a % K] = yr[offs[j]: offs[j] + len(a)]
    out += (top[:, :, None].astype(np.float32) * ytk).sum(axis=1)
    return out.astype(np.float32)



# revision 6
# speedup vs baseline: 1.1006x; 1.1006x over previous
"""DeepSeek-V3-style MoE layer on 8 Trainium2 NeuronCores.

Strategy (uniform expert-parallel, shared expert folded into routed path):
  - Router (sigmoid over rand_logits, top-4, capacity drop) runs on host:
    it is O(T*E) index math that determines the dispatch, i.e. the sharding.
  - The shared expert (MS = 2816 = 2 x 1408) is exactly two standard-shaped
    experts (D -> M SwiGLU -> D). Each half is token-split 4 ways: cores 0-3
    run half 0, cores 4-7 run half 1, each over a 512-token quarter. This
    removes the 352->384 intermediate padding the sliced layout needed.
  - The 32 routed experts are placed one per (core, segment) cell on a
    4-segment grid; segment capacities are the max routed load in each
    sorted octile (SPMD: every core runs the identical instruction stream).
  - y is written back as [d-tile, 128, tok] fp16 (no on-chip transpose);
    the host transposes, applies routing weights, and scatter-adds.

All matmuls run on the tensor engine with fp16 operands (fp32 PSUM).
"""

import functools
import os
import sys
import time

import numpy as np

for _p in ('/opt/trn_rl_repo', '/root/.axon_site/_ro/trn_rl_repo'):
    if os.path.isdir(_p) and _p not in sys.path:
        sys.path.insert(0, _p)

import concourse.bass as bass  # noqa: F401  (AP helpers)
import concourse.tile as tile
from concourse import bacc, mybir
from concourse.bass_utils import run_bass_kernel_spmd

# ---- problem config (hardcoded from spec) ----
T = 2048
D = 2048          # hidden
M = 1408          # expert intermediate
E = 32            # experts
K = 4             # top_k
CAP = 512         # per-expert capacity
ROUTE_SCALE = 2.5
N_CORES = 8
NSEG = 5          # 1 shared-half segment + 4 routed segments
KT = D // 128     # 16 contraction tiles over hidden
MT = M // 128     # 11 intermediate tiles
SH_CAP = T // 4   # 512 tokens per shared-half quarter

DT, NP_DT = mybir.dt.float16, np.float16
F32 = mybir.dt.float32
SILU = mybir.ActivationFunctionType.Silu


# --------------------------------------------------------------------------
# host-side routing
# --------------------------------------------------------------------------

def _route(rand_logits, expert_bias):
    scores = (1.0 / (1.0 + np.exp(-rand_logits.astype(np.float32)))).astype(np.float32)
    biased = scores + expert_bias[None, :]
    idx = np.argsort(-biased, axis=1, kind="stable")[:, :K]          # [T, K]
    top = np.take_along_axis(scores, idx, axis=1)
    top = top / (top.sum(-1, keepdims=True) + 1e-20) * ROUTE_SCALE   # [T, K]

    flat_e = idx.reshape(-1)
    order = np.argsort(flat_e, kind="stable")                        # assignment ids by expert
    counts = np.bincount(flat_e, minlength=E)
    kept = np.minimum(counts, CAP)
    starts = np.concatenate([[0], np.cumsum(counts)])[:E]
    assigns = [order[starts[e]: starts[e] + kept[e]] for e in range(E)]
    return top, assigns, kept


def _placement(kept):
    """Experts -> (segment, core) grid; segment cap = max load in its octile."""
    rank = np.argsort(-kept, kind="stable")
    slots = rank.reshape(4, N_CORES)                 # routed segment s, core c
    caps = (SH_CAP,) + tuple(int(kept[slots[s][0]]) for s in range(4))
    return slots, caps


# --------------------------------------------------------------------------
# device program
# --------------------------------------------------------------------------

# slot = group of segments sharing one ht/y tensor (keeps DMA runs >= 512B)
SLOT_SEGS = ([0], [1, 2], [3, 4])


@functools.lru_cache(maxsize=4)
def _program(caps):
    capsum = sum(caps)
    offs = [0]
    for c in caps:
        offs.append(offs[-1] + c)

    nc = bacc.Bacc("TRN2", target_bir_lowering=False, debug=False,
                   num_devices=N_CORES)
    ap = {}
    ap["xt"] = nc.dram_tensor("xt", [KT, 128, capsum], DT, kind="ExternalInput").ap()
    ap["wg"] = nc.dram_tensor("wg", [NSEG, MT, 128, KT * 128], DT, kind="ExternalInput").ap()
    ap["wu"] = nc.dram_tensor("wu", [NSEG, MT, 128, KT * 128], DT, kind="ExternalInput").ap()
    ap["wd"] = nc.dram_tensor("wd", [NSEG, MT, 128, D], DT, kind="ExternalInput").ap()
    for si, segs in enumerate(SLOT_SEGS):
        w = sum(caps[s] for s in segs)
        ap[f"yr{si}"] = nc.dram_tensor(f"yr{si}", [KT, 128, w], DT,
                                       kind="ExternalOutput").ap()

    with tile.TileContext(nc) as tc:
        with tc.tile_pool(name="xtp", bufs=1) as xtp, \
             tc.tile_pool(name="wp", bufs=8) as wp, \
             tc.tile_pool(name="hp", bufs=2) as hp, \
             tc.tile_pool(name="wdp", bufs=6) as wdp, \
             tc.tile_pool(name="actp", bufs=3) as actp, \
             tc.tile_pool(name="obp", bufs=4) as obp, \
             tc.tile_pool(name="psgu", bufs=6, space="PSUM") as psgu, \
             tc.tile_pool(name="psy", bufs=2, space="PSUM") as psy:

            xt_sb = xtp.tile([128, KT, capsum], DT, name="xt_sb")

            for si, segs in enumerate(SLOT_SEGS):
                soff = offs[segs[0]]                      # global col offset
                scap = sum(caps[s] for s in segs)
                # local (offset, cap) of each segment within the slot
                lseg = []
                o = 0
                for s in segs:
                    lseg.append((s, o, caps[s]))
                    o += caps[s]

                ht = hp.tile([128, MT, scap], DT, name="ht", tag="ht")
                for m in range(MT):
                    wpairs = []
                    for s, lo, c in lseg:
                        wg_sb = wp.tile([128, KT * 128], DT, name="wg_sb", tag="w")
                        wu_sb = wp.tile([128, KT * 128], DT, name="wu_sb", tag="w")
                        if si == 0 and m == 0:
                            # cold start: need-ordered chunks; token tiles as
                            # singles -> pair -> quads to dodge the per-DMA
                            # descriptor cadence while feeding the k-loop
                            def xtld(t0, t1):
                                nc.sync.dma_start(
                                    xt_sb[:, t0:t1, :SH_CAP],
                                    ap["xt"][t0:t1].transpose([1, 0, 2])[:, :, :SH_CAP])
                            nc.sync.dma_start(wg_sb[:, :512], ap["wg"][s, m, :, :512])
                            nc.sync.dma_start(wu_sb[:, :512], ap["wu"][s, m, :, :512])
                            xtld(0, 1)
                            xtld(1, 2)
                            nc.sync.dma_start(wg_sb[:, 512:1024], ap["wg"][s, m, :, 512:1024])
                            nc.sync.dma_start(wu_sb[:, 512:1024], ap["wu"][s, m, :, 512:1024])
                            xtld(2, 4)
                            xtld(4, 8)
                            nc.sync.dma_start(wg_sb[:, 1024:], ap["wg"][s, m, :, 1024:])
                            nc.sync.dma_start(wu_sb[:, 1024:], ap["wu"][s, m, :, 1024:])
                            xtld(8, 12)
                            xtld(12, 16)
                        else:
                            # stream gate/up halves interleaved so the k-loop
                            # can start before the full m-tile lands
                            nc.sync.dma_start(wg_sb[:, :1024], ap["wg"][s, m, :, :1024])
                            nc.sync.dma_start(wu_sb[:, :1024], ap["wu"][s, m, :, :1024])
                            nc.sync.dma_start(wg_sb[:, 1024:], ap["wg"][s, m, :, 1024:])
                            nc.sync.dma_start(wu_sb[:, 1024:], ap["wu"][s, m, :, 1024:])
                        wpairs.append((wg_sb, wu_sb))
                    if si == 0 and 2 <= m < 10:
                        # backfill routed token columns (needed from slot 1 on)
                        for t in range(2 * (m - 2), 2 * (m - 1)):
                            nc.sync.dma_start(xt_sb[:, t, SH_CAP:],
                                              ap["xt"][t][:, SH_CAP:])

                    for (s, lo, c), (wg_sb, wu_sb) in zip(lseg, wpairs):
                        psg = psgu.tile([128, c], F32, name="psg", tag="psgu")
                        psu = psgu.tile([128, c], F32, name="psu", tag="psgu")
                        rhs = xt_sb[:, :, offs[s]: offs[s] + c]
                        for t in range(KT):
                            nc.tensor.matmul(psg[:], wg_sb[:, t * 128:(t + 1) * 128],
                                             rhs[:, t, :], start=(t == 0), stop=(t == KT - 1))
                            nc.tensor.matmul(psu[:], wu_sb[:, t * 128:(t + 1) * 128],
                                             rhs[:, t, :], start=(t == 0), stop=(t == KT - 1))
                        sact = actp.tile([128, c], F32, name="sact", tag="act")
                        nc.scalar.activation(sact[:], psg[:], SILU)
                        nc.vector.tensor_mul(ht[:, m, lo:lo + c], sact[:], psu[:])

                # down-projection: out stays [d-part, tok]; host re-layouts
                for g in range(4):
                    wds = []
                    for s, lo, c in lseg:
                        wd_g = wdp.tile([128, MT, 512], DT, name="wd_g", tag="wd")
                        nc.sync.dma_start(
                            wd_g[:],
                            ap["wd"][s].transpose([1, 0, 2])[:, :, g * 512:(g + 1) * 512])
                        wds.append(wd_g)
                    for k in range(4):
                        ob = obp.tile([128, scap], DT, name="ob", tag="ob")
                        for (s, lo, c), wd_g in zip(lseg, wds):
                            ps = psy.tile([128, c], F32, name="ps_y", tag="psy")
                            for m in range(MT):
                                nc.tensor.matmul(ps[:], wd_g[:, m, k * 128:(k + 1) * 128],
                                                 ht[:, m, lo:lo + c],
                                                 start=(m == 0), stop=(m == MT - 1))
                            nc.vector.tensor_copy(ob[:, lo:lo + c], ps[:])
                        nc.sync.dma_start(ap[f"yr{si}"][g * 4 + k], ob[:])
    nc.compile()
    return nc


# --------------------------------------------------------------------------
# host-side packing + combine
# --------------------------------------------------------------------------

def _pack_gu(w):
    # [D, M] -> [MT, 128(k-part), KT*128] stationary-ready layout
    return np.ascontiguousarray(
        w.reshape(KT, 128, MT, 128).transpose(2, 1, 0, 3).reshape(MT, 128, KT * 128))


def kernel(**inputs):
    x = np.asarray(inputs["x"], np.float32)
    rand_logits = np.asarray(inputs["rand_logits"], np.float32)
    expert_bias = np.asarray(inputs["expert_bias"], np.float32)
    wg = np.asarray(inputs["w_gate"], np.float32)
    wu = np.asarray(inputs["w_up"], np.float32)
    wd = np.asarray(inputs["w_down"], np.float32)
    swg = np.asarray(inputs["sw_gate"], np.float32)
    swu = np.asarray(inputs["sw_up"], np.float32)
    swd = np.asarray(inputs["sw_down"], np.float32)

    top, assigns, kept = _route(rand_logits, expert_bias)
    slots, caps = _placement(kept)
    capsum = sum(caps)
    offs = np.concatenate([[0], np.cumsum(caps)]).astype(int)

    global _last_caps
    _last_caps = caps
    t0 = time.time()
    nc = _program(caps)
    t1 = time.time()

    xT = np.ascontiguousarray(x.T.astype(NP_DT))                    # [D, T]

    in_maps = []
    for c in range(N_CORES):
        half, quarter = c // 4, c % 4
        xt = np.zeros((D, capsum), NP_DT)
        xt[:, :SH_CAP] = xT[:, quarter * SH_CAP:(quarter + 1) * SH_CAP]
        for s in range(4):
            e = slots[s][c]
            tok = assigns[e] // K
            if len(tok):
                xt[:, offs[s + 1]: offs[s + 1] + len(tok)] = xT[:, tok]

        seg_w = [(swg[:, half * M:(half + 1) * M],
                  swu[:, half * M:(half + 1) * M],
                  swd[half * M:(half + 1) * M, :])]
        seg_w += [(wg[slots[s][c]], wu[slots[s][c]], wd[slots[s][c]])
                  for s in range(4)]
        wgx = np.stack([_pack_gu(g) for g, _, _ in seg_w])
        wux = np.stack([_pack_gu(u) for _, u, _ in seg_w])
        wdx = np.stack([d.reshape(MT, 128, D) for _, _, d in seg_w])

        in_maps.append({
            "xt": xt.reshape(KT, 128, capsum),
            "wg": wgx.astype(NP_DT),
            "wu": wux.astype(NP_DT),
            "wd": wdx.astype(NP_DT),
        })

    t2 = time.time()
    res = run_bass_kernel_spmd(nc, in_maps, core_ids=list(range(N_CORES)))
    t3 = time.time()
    if os.environ.get("BASSMOE_VERBOSE"):
        print(f"[kernel] program build {t1 - t0:.2f}s  pack {t2 - t1:.2f}s  "
              f"device run {t3 - t2:.2f}s", file=sys.stderr)
    outs = res.results

    # slot tensors -> per-core [tok, D] blocks
    def unpack(arr):                                    # [KT, 128, w] -> [w, D]
        return arr.transpose(2, 0, 1).reshape(arr.shape[2], D).astype(np.float32)

    out = np.zeros((T, D), np.float32)
    ytk = np.zeros((T, K, D), np.float32)
    for c in range(N_CORES):
        quarter = c % 4
        ysh = unpack(outs[c]["yr0"])                    # shared-half quarter
        out[quarter * SH_CAP:(quarter + 1) * SH_CAP] += ysh
        y1 = unpack(outs[c]["yr1"])
        y2 = unpack(outs[c]["yr2"])
        lofs = (0, 0, caps[1], 0, caps[3])
        ys = (None, y1, y1, y2, y2)
        for s in range(4):
            e = slots[s][c]
            a = assigns[e]
            if len(a):
                ytk[a // K, a % K] = ys[s + 1][lofs[s + 1]: lofs[s + 1] + len(a)]
    out += (top[:, :, None].astype(np.float32) * ytk).sum(axis=1)
    return out.astype(np.float32)


# revision 14
# speedup vs baseline: 1.1042x; 1.0033x over previous
"""DeepSeek-V3-style MoE layer on 8 Trainium2 NeuronCores.

Strategy (uniform expert-parallel, shared expert folded into routed path):
  - Router (sigmoid over rand_logits, top-4, capacity drop) runs on host:
    it is O(T*E) index math that determines the dispatch, i.e. the sharding.
  - The shared expert (MS = 2816 = 2 x 1408) is exactly two standard-shaped
    experts (D -> M SwiGLU -> D). Each half is token-split 4 ways: cores 0-3
    run half 0, cores 4-7 run half 1, each over a 512-token quarter. This
    removes the 352->384 intermediate padding the sliced layout needed.
  - The 32 routed experts are placed one per (core, segment) cell on a
    4-segment grid; segment capacities are the max routed load in each
    sorted octile (SPMD: every core runs the identical instruction stream).
  - y is written back as [d-tile, 128, tok] fp16 (no on-chip transpose);
    the host transposes, applies routing weights, and scatter-adds.

All matmuls run on the tensor engine with fp16 operands (fp32 PSUM).
"""

import functools
import os
import sys
import time

import numpy as np

for _p in ('/opt/trn_rl_repo', '/root/.axon_site/_ro/trn_rl_repo'):
    if os.path.isdir(_p) and _p not in sys.path:
        sys.path.insert(0, _p)

import concourse.bass as bass  # noqa: F401  (AP helpers)
import concourse.tile as tile
from concourse import bacc, mybir
from concourse.bass_utils import run_bass_kernel_spmd

# ---- problem config (hardcoded from spec) ----
T = 2048
D = 2048          # hidden
M = 1408          # expert intermediate
E = 32            # experts
K = 4             # top_k
CAP = 512         # per-expert capacity
ROUTE_SCALE = 2.5
N_CORES = 8
NSEG = 5          # 1 shared-half segment + 4 routed segments
KT = D // 128     # 16 contraction tiles over hidden
MT = M // 128     # 11 intermediate tiles
SH_CAP = T // 4   # 512 tokens per shared-half quarter

DT, NP_DT = mybir.dt.float16, np.float16
F32 = mybir.dt.float32
SILU = mybir.ActivationFunctionType.Silu


# --------------------------------------------------------------------------
# host-side routing
# --------------------------------------------------------------------------

def _route(rand_logits, expert_bias):
    scores = (1.0 / (1.0 + np.exp(-rand_logits.astype(np.float32)))).astype(np.float32)
    biased = scores + expert_bias[None, :]
    idx = np.argsort(-biased, axis=1, kind="stable")[:, :K]          # [T, K]
    top = np.take_along_axis(scores, idx, axis=1)
    top = top / (top.sum(-1, keepdims=True) + 1e-20) * ROUTE_SCALE   # [T, K]

    flat_e = idx.reshape(-1)
    order = np.argsort(flat_e, kind="stable")                        # assignment ids by expert
    counts = np.bincount(flat_e, minlength=E)
    kept = np.minimum(counts, CAP)
    starts = np.concatenate([[0], np.cumsum(counts)])[:E]
    assigns = [order[starts[e]: starts[e] + kept[e]] for e in range(E)]
    return top, assigns, kept


def _placement(kept):
    """Experts -> (segment, core) grid; segment cap = max load in its octile."""
    rank = np.argsort(-kept, kind="stable")
    slots = rank.reshape(4, N_CORES)                 # routed segment s, core c
    caps = (SH_CAP,) + tuple(int(kept[slots[s][0]]) for s in range(4))
    return slots, caps


# --------------------------------------------------------------------------
# device program
# --------------------------------------------------------------------------

# slot = group of segments sharing one ht/y tensor (keeps DMA runs >= 512B)
SLOT_SEGS = ([0], [1, 2], [3, 4])


@functools.lru_cache(maxsize=4)
def _program(caps):
    capsum = sum(caps)
    offs = [0]
    for c in caps:
        offs.append(offs[-1] + c)

    nc = bacc.Bacc("TRN2", target_bir_lowering=False, debug=False,
                   num_devices=N_CORES)
    ap = {}
    ap["xt"] = nc.dram_tensor("xt", [KT, 128, capsum], DT, kind="ExternalInput").ap()
    ap["wg"] = nc.dram_tensor("wg", [NSEG, MT, 128, KT * 128], DT, kind="ExternalInput").ap()
    ap["wu"] = nc.dram_tensor("wu", [NSEG, MT, 128, KT * 128], DT, kind="ExternalInput").ap()
    ap["wd"] = nc.dram_tensor("wd", [NSEG, MT, 128, D], DT, kind="ExternalInput").ap()
    for si, segs in enumerate(SLOT_SEGS):
        w = sum(caps[s] for s in segs)
        ap[f"yr{si}"] = nc.dram_tensor(f"yr{si}", [KT, 128, w], DT,
                                       kind="ExternalOutput").ap()


    with tile.TileContext(nc) as tc:
        with tc.tile_pool(name="xtp", bufs=1) as xtp, \
             tc.tile_pool(name="wp", bufs=8) as wp, \
             tc.tile_pool(name="hp", bufs=2) as hp, \
             tc.tile_pool(name="wdp", bufs=6) as wdp, \
             tc.tile_pool(name="actp", bufs=3) as actp, \
             tc.tile_pool(name="obp", bufs=4) as obp, \
             tc.tile_pool(name="psgu", bufs=6, space="PSUM") as psgu, \
             tc.tile_pool(name="psy", bufs=2, space="PSUM") as psy:

            xt_sb = xtp.tile([128, KT, capsum], DT, name="xt_sb")

            for si, segs in enumerate(SLOT_SEGS):
                soff = offs[segs[0]]                      # global col offset
                scap = sum(caps[s] for s in segs)
                # local (offset, cap) of each segment within the slot
                lseg = []
                o = 0
                for s in segs:
                    lseg.append((s, o, caps[s]))
                    o += caps[s]

                ht = hp.tile([128, MT, scap], DT, name="ht", tag="ht")
                for m in range(MT):
                    wpairs = []
                    for s, lo, c in lseg:
                        wg_sb = wp.tile([128, KT * 128], DT, name="wg_sb", tag="w")
                        wu_sb = wp.tile([128, KT * 128], DT, name="wu_sb", tag="w")
                        if si == 0 and m == 0:
                            # cold start: need-ordered chunks; token tiles as
                            # singles -> pair -> quads to dodge the per-DMA
                            # descriptor cadence while feeding the k-loop
                            def xtld(t0, t1):
                                nc.sync.dma_start(
                                    xt_sb[:, t0:t1, :SH_CAP],
                                    ap["xt"][t0:t1].transpose([1, 0, 2])[:, :, :SH_CAP])
                            nc.sync.dma_start(wg_sb[:, :512], ap["wg"][s, m, :, :512])
                            xtld(0, 2)
                            nc.sync.dma_start(wu_sb[:, :512], ap["wu"][s, m, :, :512])
                            xtld(2, 4)
                            nc.sync.dma_start(wg_sb[:, 512:1024], ap["wg"][s, m, :, 512:1024])
                            nc.sync.dma_start(wu_sb[:, 512:1024], ap["wu"][s, m, :, 512:1024])
                            xtld(4, 6)
                            nc.sync.dma_start(wg_sb[:, 1024:], ap["wg"][s, m, :, 1024:])
                            nc.sync.dma_start(wu_sb[:, 1024:], ap["wu"][s, m, :, 1024:])
                            xtld(6, 8)
                            xtld(8, 10)
                            xtld(10, 12)
                            xtld(12, 14)
                            xtld(14, 16)
                        else:
                            # stream gate/up halves interleaved so the k-loop
                            # can start before the full m-tile lands
                            nc.sync.dma_start(wg_sb[:, :1024], ap["wg"][s, m, :, :1024])
                            nc.sync.dma_start(wu_sb[:, :1024], ap["wu"][s, m, :, :1024])
                            nc.sync.dma_start(wg_sb[:, 1024:], ap["wg"][s, m, :, 1024:])
                            nc.sync.dma_start(wu_sb[:, 1024:], ap["wu"][s, m, :, 1024:])
                        wpairs.append((wg_sb, wu_sb))
                    if si == 0 and 2 <= m < 10:
                        # backfill routed token columns (needed from slot 1 on)
                        for t in range(2 * (m - 2), 2 * (m - 1)):
                            nc.sync.dma_start(xt_sb[:, t, SH_CAP:],
                                              ap["xt"][t][:, SH_CAP:])

                    for (s, lo, c), (wg_sb, wu_sb) in zip(lseg, wpairs):
                        psg = psgu.tile([128, c], F32, name="psg", tag="psgu")
                        psu = psgu.tile([128, c], F32, name="psu", tag="psgu")
                        rhs = xt_sb[:, :, offs[s]: offs[s] + c]
                        for t in range(KT):
                            nc.tensor.matmul(psg[:], wg_sb[:, t * 128:(t + 1) * 128],
                                             rhs[:, t, :], start=(t == 0), stop=(t == KT - 1))
                            nc.tensor.matmul(psu[:], wu_sb[:, t * 128:(t + 1) * 128],
                                             rhs[:, t, :], start=(t == 0), stop=(t == KT - 1))
                        sact = actp.tile([128, c], F32, name="sact", tag="act")
                        nc.scalar.activation(sact[:], psg[:], SILU)
                        nc.vector.tensor_mul(ht[:, m, lo:lo + c], sact[:], psu[:])

                # down-projection: out stays [d-part, tok]; host re-layouts
                for g in range(4):
                    wds = []
                    for s, lo, c in lseg:
                        wd_g = wdp.tile([128, MT, 512], DT, name="wd_g", tag="wd")
                        nc.sync.dma_start(
                            wd_g[:],
                            ap["wd"][s].transpose([1, 0, 2])[:, :, g * 512:(g + 1) * 512])
                        wds.append(wd_g)
                    for k in range(4):
                        ob = obp.tile([128, scap], DT, name="ob", tag="ob")
                        for (s, lo, c), wd_g in zip(lseg, wds):
                            ps = psy.tile([128, c], F32, name="ps_y", tag="psy")
                            for m in range(MT):
                                nc.tensor.matmul(ps[:], wd_g[:, m, k * 128:(k + 1) * 128],
                                                 ht[:, m, lo:lo + c],
                                                 start=(m == 0), stop=(m == MT - 1))
                            nc.vector.tensor_copy(ob[:, lo:lo + c], ps[:])
                        nc.sync.dma_start(ap[f"yr{si}"][g * 4 + k], ob[:])
    nc.compile()
    return nc


# --------------------------------------------------------------------------
# host-side packing + combine
# --------------------------------------------------------------------------

def _pack_gu(w):
    # [D, M] -> [MT, 128(k-part), KT*128] stationary-ready layout
    return np.ascontiguousarray(
        w.reshape(KT, 128, MT, 128).transpose(2, 1, 0, 3).reshape(MT, 128, KT * 128))


def kernel(**inputs):
    x = np.asarray(inputs["x"], np.float32)
    rand_logits = np.asarray(inputs["rand_logits"], np.float32)
    expert_bias = np.asarray(inputs["expert_bias"], np.float32)
    wg = np.asarray(inputs["w_gate"], np.float32)
    wu = np.asarray(inputs["w_up"], np.float32)
    wd = np.asarray(inputs["w_down"], np.float32)
    swg = np.asarray(inputs["sw_gate"], np.float32)
    swu = np.asarray(inputs["sw_up"], np.float32)
    swd = np.asarray(inputs["sw_down"], np.float32)

    top, assigns, kept = _route(rand_logits, expert_bias)
    slots, caps = _placement(kept)
    capsum = sum(caps)
    offs = np.concatenate([[0], np.cumsum(caps)]).astype(int)

    global _last_caps
    _last_caps = caps
    t0 = time.time()
    nc = _program(caps)
    t1 = time.time()

    xT = np.ascontiguousarray(x.T.astype(NP_DT))                    # [D, T]

    in_maps = []
    for c in range(N_CORES):
        half, quarter = c // 4, c % 4
        xt = np.zeros((D, capsum), NP_DT)
        xt[:, :SH_CAP] = xT[:, quarter * SH_CAP:(quarter + 1) * SH_CAP]
        for s in range(4):
            e = slots[s][c]
            tok = assigns[e] // K
            if len(tok):
                xt[:, offs[s + 1]: offs[s + 1] + len(tok)] = xT[:, tok]

        seg_w = [(swg[:, half * M:(half + 1) * M],
                  swu[:, half * M:(half + 1) * M],
                  swd[half * M:(half + 1) * M, :])]
        seg_w += [(wg[slots[s][c]], wu[slots[s][c]], wd[slots[s][c]])
                  for s in range(4)]
        wgx = np.stack([_pack_gu(g) for g, _, _ in seg_w])
        wux = np.stack([_pack_gu(u) for _, u, _ in seg_w])
        wdx = np.stack([d.reshape(MT, 128, D) for _, _, d in seg_w])

        in_maps.append({
            "xt": xt.reshape(KT, 128, capsum),
            "wg": wgx.astype(NP_DT),
            "wu": wux.astype(NP_DT),
            "wd": wdx.astype(NP_DT),
        })

    t2 = time.time()
    res = run_bass_kernel_spmd(nc, in_maps, core_ids=list(range(N_CORES)))
    t3 = time.time()
    if os.environ.get("BASSMOE_VERBOSE"):
        print(f"[kernel] program build {t1 - t0:.2f}s  pack {t2 - t1:.2f}s  "
              f"device run {t3 - t2:.2f}s", file=sys.stderr)
    outs = res.results

    # slot tensors -> per-core [tok, D] blocks
    def unpack(arr):                                    # [KT, 128, w] -> [w, D]
        return arr.transpose(2, 0, 1).reshape(arr.shape[2], D).astype(np.float32)

    out = np.zeros((T, D), np.float32)
    ytk = np.zeros((T, K, D), np.float32)
    for c in range(N_CORES):
        quarter = c % 4
        ysh = unpack(outs[c]["yr0"])                    # shared-half quarter
        out[quarter * SH_CAP:(quarter + 1) * SH_CAP] += ysh
        y1 = unpack(outs[c]["yr1"])
        y2 = unpack(outs[c]["yr2"])
        lofs = (0, 0, caps[1], 0, caps[3])
        ys = (None, y1, y1, y2, y2)
        for s in range(4):
            e = slots[s][c]
            a = assigns[e]
            if len(a):
                ytk[a // K, a % K] = ys[s + 1][lofs[s + 1]: lofs[s + 1] + len(a)]
    out += (top[:, :, None].astype(np.float32) * ytk).sum(axis=1)
    return out.astype(np.float32)
